# revision 3
# baseline (speedup 1.0000x reference)
"""Causal depthwise conv (B=8, L=4096, D=1024, K=15) on 8 TRN2 NeuronCores.

Sharding: channels split across the 8 cores (128 channels each); every core
processes all 8 batch sequences for its channel slice. Host re-lays-out x to
[channels, batch, time] fp16 so on-chip tiles have channels on SBUF
partitions and time on the free dimension; tap shifts are free-dim offsets.

Per-core engine split of the 15 taps (fp16 compute, fp32 PSUM), balanced so
PE / DVE / ScalarE each carry ~17.3 us per batch of 4096:
  - TensorE (9 taps {0,1,3,5,7,9,11,13,14}): diagonal-weight matmuls
    accumulating in PSUM, 2048-wide halves.
  - DVE: tensor_scalar_mul products for taps {2,4} (4x packed mode; offsets
    must stay even) plus the second half of tap 6, then 6 tensor_tensor adds
    (2x mode) folding products and the PSUM bridge.
  - ScalarE: activation-mul products for taps {8,10,12} + first half of
    tap 6, and the PSUM->SBUF fp16 bridge copies.
Last batch runs a chunked epilogue (1024-wide bridge/merge/store) to cut the
serial tail. Output written as fp16; host upcasts to fp32.
"""

from contextlib import ExitStack

import numpy as np

import concourse.bacc as bacc
import concourse.tile as tile
from concourse import mybir
from concourse.bass_utils import run_bass_kernel_spmd

F32 = mybir.dt.float32
F16 = mybir.dt.float16
F16NP = np.float16

B = 8
L = 4096
D = 1024
K = 15
NCORES = 8
CPC = D // NCORES  # channels per core = 128
LP = L + K - 1  # 4110

DVE_MUL_TAPS = [2, 4]  # full products on DVE (even offsets -> 4x mode)
SPLIT_TAP = 6  # first half ScalarE, second half DVE
SC_MUL_TAPS = [8, 10, 12]
PE_TAPS = [0, 1, 3, 5, 7, 9, 11, 13, 14]

_compiled_nc = None
_last_in_maps = None


def _build_nc():
    nc = bacc.Bacc(
        "TRN2",
        target_bir_lowering=False,
        debug=False,
        enable_asserts=True,
        num_devices=NCORES,
    )
    x = nc.dram_tensor("x", [CPC, B, LP], F16, kind="ExternalInput").ap()
    diag = nc.dram_tensor("diag", [len(PE_TAPS), CPC, CPC], F16, kind="ExternalInput").ap()
    w = nc.dram_tensor("w", [CPC, 16], F32, kind="ExternalInput").ap()
    out = nc.dram_tensor("out", [CPC, B, L], F16, kind="ExternalOutput").ap()

    add = mybir.AluOpType.add

    with tile.TileContext(nc) as tc, ExitStack() as ctx:
        const_pool = ctx.enter_context(tc.tile_pool(name="const", bufs=1))
        xp = ctx.enter_context(tc.tile_pool(name="xp", bufs=3))
        prodp = ctx.enter_context(tc.tile_pool(name="prodp", bufs=8))
        sump = ctx.enter_context(tc.tile_pool(name="sump", bufs=6))
        accp = ctx.enter_context(tc.tile_pool(name="accp", bufs=2))
        op = ctx.enter_context(tc.tile_pool(name="op", bufs=2))
        pp = ctx.enter_context(tc.tile_pool(name="pp", bufs=2, space="PSUM"))

        wt = const_pool.tile([CPC, 16], F32, tag="w")
        nc.scalar.dma_start(wt[:], w[:])
        dg = const_pool.tile([CPC, len(PE_TAPS) * CPC], F16, tag="diag")
        for j in range(len(PE_TAPS)):
            nc.scalar.dma_start(dg[:, j * CPC : (j + 1) * CPC], diag[j])

        for b in range(B):
            xt = xp.tile([CPC, LP], F16, tag="x", name=f"x_{b}")
            if b == 0:
                # small first piece so the first matmuls start ASAP
                cuts = [0, 600, 2400, LP]
            else:
                cuts = [0, LP // 2, LP]
            for s0, s1 in zip(cuts[:-1], cuts[1:]):
                nc.sync.dma_start(xt[:, s0:s1], x[:, b, s0:s1])

            prods = {}

            # ScalarE products: taps {8,10,12} full + tap 6 first half
            for k in SC_MUL_TAPS:
                pt = prodp.tile([CPC, L], F16, tag="prod", name=f"sp_{b}_{k}")
                nc.scalar.mul(pt[:], xt[:, k : k + L], wt[:, k : k + 1])
                prods[k] = pt
            k = SPLIT_TAP
            pt6 = prodp.tile([CPC, L], F16, tag="prod", name=f"sp_{b}_{k}")
            nc.scalar.mul(pt6[:, 0:2048], xt[:, k : k + 2048], wt[:, k : k + 1])
            prods[k] = pt6

            # DVE products: taps {2,4} full + tap 6 second half (even offsets)
            for k in DVE_MUL_TAPS:
                pt = prodp.tile([CPC, L], F16, tag="prod", name=f"dp_{b}_{k}")
                nc.vector.tensor_scalar_mul(pt[:], xt[:, k : k + L], wt[:, k : k + 1])
                prods[k] = pt
            k = SPLIT_TAP
            nc.vector.tensor_scalar_mul(
                pt6[:, 2048:4096], xt[:, k + 2048 : k + L], wt[:, k : k + 1]
            )

            # TensorE: 9 taps into PSUM, two 2048-wide halves
            last = b == B - 1
            acc = accp.tile([CPC, L], F16, tag="acc", name=f"acc_{b}")
            for h in range(2):
                t0 = h * 2048
                ps = pp.tile([CPC, 2048], F32, tag="ps", name=f"ps_{b}_{h}")
                for ji, k in enumerate(PE_TAPS):
                    for q in range(4):
                        nc.tensor.matmul(
                            ps[:, q * 512 : (q + 1) * 512],
                            dg[:, ji * CPC : (ji + 1) * CPC],
                            xt[:, t0 + k + q * 512 : t0 + k + (q + 1) * 512],
                            start=(ji == 0),
                            stop=(ji == len(PE_TAPS) - 1),
                        )
                # ScalarE bridge: PSUM fp32 -> SBUF fp16
                if last:
                    for q in range(2):
                        nc.scalar.copy(
                            acc[:, t0 + q * 1024 : t0 + (q + 1) * 1024],
                            ps[:, q * 1024 : (q + 1) * 1024],
                        )
                else:
                    nc.scalar.copy(acc[:, t0 : t0 + 2048], ps[:])

            # DVE folds: 5 product folds, then merge with the PE bridge
            order = [2, 4, 6, 8, 10, 12]
            s = prods[order[0]]
            for i, k in enumerate(order[1:]):
                dst = sump.tile([CPC, L], F16, tag="sum", name=f"s_{b}_{i}")
                nc.vector.tensor_tensor(dst[:], prods[k][:], s[:], add)
                s = dst
            ot = op.tile([CPC, L], F16, tag="osb", name=f"o_{b}")
            if last:
                for c in range(4):
                    sl = slice(c * 1024, (c + 1) * 1024)
                    nc.vector.tensor_tensor(ot[:, sl], s[:, sl], acc[:, sl], add)
                    nc.scalar.dma_start(out[:, b, sl], ot[:, sl])
            else:
                nc.vector.tensor_tensor(ot[:], s[:], acc[:], add)
                nc.scalar.dma_start(out[:, b, :], ot[:])

    nc.compile()
    return nc


def kernel(x: np.ndarray, weight: np.ndarray) -> np.ndarray:
    """x: [8, 4096, 1024] fp32, weight: [15, 1, 1024] fp32 ->
    [8, 4096, 1024] fp32 causal depthwise conv."""
    global _compiled_nc
    if _compiled_nc is None:
        _compiled_nc = _build_nc()
    nc = _compiled_nc

    x = np.ascontiguousarray(x, dtype=np.float32)
    wk = np.ascontiguousarray(weight, dtype=np.float32).reshape(K, D)
    x16 = x.astype(F16NP)
    wk16 = wk.astype(F16NP)

    in_maps = []
    for c in range(NCORES):
        sl = slice(c * CPC, (c + 1) * CPC)
        xpad = np.zeros((CPC, B, LP), dtype=F16NP)
        xpad[:, :, K - 1 :] = x16[:, :, sl].transpose(2, 0, 1)
        dgc = np.zeros((len(PE_TAPS), CPC, CPC), dtype=F16NP)
        didx = np.arange(CPC)
        for j, k in enumerate(PE_TAPS):
            dgc[j, didx, didx] = wk16[k, sl]
        wt = np.zeros((CPC, 16), dtype=np.float32)
        wt[:, :K] = wk[:, sl].T
        in_maps.append({"x": xpad, "diag": dgc, "w": wt})

    global _last_in_maps
    _last_in_maps = in_maps
    res = run_bass_kernel_spmd(nc, in_maps, list(range(NCORES)))

    out = np.empty((B, L, D), dtype=np.float32)
    for c in range(NCORES):
        sl = slice(c * CPC, (c + 1) * CPC)
        out[:, :, sl] = res.results[c]["out"].transpose(1, 2, 0).astype(np.float32)
    return out
